# revision 37
# baseline (speedup 1.0000x reference)
"""Trainium2 Bass kernel for a dense transformer block (MLA attention + SwiGLU MLP).

Problem: B=2, T=2048, D=2048, HQ=16, HKV=4, DH=128, RQ=512, RKV=256, DFF=8192.

Sharding: sequence-parallel over 8 cores. Core c (batch b=c//4, lane j=c%4)
owns 4 query chunks of 128 tokens: chunk i = tokens [512i+128j, 512i+128(j+1)).
K/V are computed for the batch's full 2048 tokens on every core (replicated,
no collectives). Causal structure is exploited at 512-key granularity: chunk i
attends exactly keys [0, 512(i+1)) — identical loop structure on every core
(SPMD), with the diagonal 512-block handled by a host-provided exp(mask)
multiplier. This removes 37.5% of attention work vs full-T.

fp8 (e4m3) with DoubleRow matmuls (K=256/instr, 2x throughput) is used where
quantization noise is damped below the error budget:
  - Q/K/V low-rank projections (noise washed out by softmax averaging)
  - P*V and softmax-sum S (P in [0,1] scaled by 16 via Exp bias=ln 16)
  - W_upB up-projection (sigmoid damps its error by ~4x)
  - the first 16 of 64 W_down row-tiles (rest bf16; full-fp8 busts the gate)
W_upA, Wo and the rest of W_down stay bf16. Measured rel err 1.61e-2 < 2e-2.

Scales: all fp8 weights x64. x/xq f8 direct. B1/A1 store 8x, QT 64x, KT 16x,
Vn 16x (with r1), pt = 16*exp(l) via bias, h2f8 4x. Descale folded into
activation scales / drain multiplies; S and P*V scales cancel in O/S except a
1/16 folded into the normalization multiply.
"""
import math
import numpy as np
import ml_dtypes

import concourse.bass as bass
import concourse.mybir as mybir
import concourse.tile as tile
from concourse import bacc
from concourse.bass_utils import run_bass_kernel_spmd
from contextlib import ExitStack

B, T, D = 2, 2048, 2048
HQ, HKV, DH = 16, 4, 128
RQ, RKV = 512, 256
DFF = 8192
EPS = 1e-5
NCORES = 8
Q = 512          # queries per core (4 chunks of 128)
P = 128
DT = D // P      # 16 d tiles
DP = DT // 2     # 8 d-tile pairs
KT = T // P      # 16 key tiles
RQT = RQ // P    # 4
RKT = RKV // P   # 2
FT = DFF // P    # 64 dff tiles
GROUP = HQ // HKV
NCH = 4          # query chunks per core

F32 = mybir.dt.float32
BF16 = mybir.dt.bfloat16
F8 = mybir.dt.float8e4
BF = ml_dtypes.bfloat16
F8NP = ml_dtypes.float8_e4m3
DR = mybir.MatmulPerfMode.DoubleRow

WS = 64.0        # fp8 weight scale
LN16 = math.log(16.0)

_CACHE = {}


def _build_nc():
    nc = bacc.Bacc("TRN2", debug=False, num_devices=NCORES)
    ap = {}
    def din(name, shape, dt):
        ap[name] = nc.dram_tensor(name, list(shape), dt, kind="ExternalInput").ap()
    din("xf8", [DP, P, 2, T], F8)          # [dp][p, s, t] = x[t, 256dp+128s+p]
    din("xqT", [D, Q], F32)                # core's 512 query tokens, transposed
    din("xqf8", [DP, P, 2, Q], F8)
    din("expm", [NCH, P, 4, 2, P], BF16)   # [i][k, t4, z, q]
    din("q1p8", [RQT, P, DT, P], F8)
    din("q2p8", [HQ, P, RQT, P], F8)
    din("k1p8", [RKT, P, DT, P], F8)
    din("k2p8", [HKV, P, RKT, P], F8)
    din("v1p8", [RKT, P, DT, P], F8)
    din("v2n8", [P, RKT, HKV * DH], F8)
    din("wop", [DT, P, DT, P], BF16)
    din("uap", [FT, P, DT, P], BF16)
    din("ubp8", [FT, P, DT, P], F8)
    din("wdp", [DT, P, FT - 24, P], BF16)
    din("wdp8", [DT, P, 24, P], F8)
    outT = nc.dram_tensor("outT", [D, Q], F32, kind="ExternalOutput").ap()

    AL = mybir.AluOpType
    AF = mybir.ActivationFunctionType

    with tile.TileContext(nc) as tc, ExitStack() as ctx:
        const = ctx.enter_context(tc.tile_pool(name="const", bufs=1))
        dram = ctx.enter_context(tc.tile_pool(name="drsc", bufs=1, space="DRAM"))

        ones = const.tile([P, 1], BF16)
        nc.vector.memset(ones, 1.0)
        ones8 = const.tile([P, 2, 32], F8)  # 32 identical cols: DR ldweights
        nc.vector.memset(ones8, 1.0)        # rejects narrower APs
        ln16c = const.tile([P, 1], F32)
        nc.vector.memset(ln16c, LN16)

        otpool = ctx.enter_context(tc.tile_pool(name="ot", bufs=1))
        pkvq = ExitStack()
        kvq = pkvq.enter_context(tc.tile_pool(name="kvq", bufs=1))
        pxf = ExitStack()
        xfpool = pxf.enter_context(tc.tile_pool(name="xf", bufs=1))
        psa_st = ExitStack()
        psA = psa_st.enter_context(tc.tile_pool(name="psA", bufs=3, space="PSUM"))

        # =============== Phase 1: load x, stats ===============
        # weights first on the scalar queue so they arrive before P2 needs them
        wp2 = ExitStack()
        wpool2 = wp2.enter_context(tc.tile_pool(name="w2", bufs=3))
        w_k1 = []
        for rt in range(RKT):
            w = wpool2.tile([P, DT, P], F8, name="wk1", tag="w8")
            nc.scalar.dma_start(out=w, in_=ap["k1p8"][rt])
            w_k1.append(w)
        w_v1 = []
        for rt in range(RKT):
            w = wpool2.tile([P, DT, P], F8, name="wv1", tag="w8")
            nc.scalar.dma_start(out=w, in_=ap["v1p8"][rt])
            w_v1.append(w)
        v2sb = kvq.tile([P, RKT, HKV * DH], F8, name="v2", tag="v2")
        nc.scalar.dma_start(out=v2sb, in_=ap["v2n8"])

        ph1 = ExitStack()
        sqpool = ph1.enter_context(tc.tile_pool(name="sq", bufs=3))
        st1 = ph1.enter_context(tc.tile_pool(name="st1", bufs=1))
        ssqp = ph1.enter_context(tc.tile_pool(name="ssqp", bufs=1, space="PSUM"))
        xqpool = ph1.enter_context(tc.tile_pool(name="xq", bufs=3))

        # xf8 tiles stay resident through P2
        xf = []
        for dp in range(DP):
            xt = xfpool.tile([P, 2, T], F8, name=f"xf{dp}", tag=f"xf{dp}")
            nc.sync.dma_start(out=xt, in_=ap["xf8"][dp])
            xf.append(xt)
        # batch-wide sum of squares -> r1p [P, KT]
        ssq = [ssqp.tile([1, 512], F32, name=f"ssq{c}", tag=f"ssq{c}") for c in range(4)]
        for dp in range(DP):
            for s in range(2):
                i = 2 * dp + s
                sq = sqpool.tile([P, T], BF16, name="sq", tag="sq")
                if i % 4 == 3:
                    nc.vector.tensor_tensor(sq, xf[dp][:, s, :], xf[dp][:, s, :],
                                            AL.mult)
                else:
                    nc.scalar.square(sq, xf[dp][:, s, :])
                for c in range(4):
                    nc.tensor.matmul(ssq[c], lhsT=ones, rhs=sq[:, c * 512:(c + 1) * 512],
                                     start=(i == 0), stop=(i == DT - 1))
        nrow = st1.tile([1, T], F32)
        for c in range(4):
            nc.scalar.activation(nrow[:, c * 512:(c + 1) * 512], ssq[c],
                                 AF.Sqrt, scale=1.0 / D)
        nd = dram.tile([1, T], F32, name="r1nd", tag="r1nd")
        nc.gpsimd.dma_start(out=nd, in_=nrow)
        np_sb = st1.tile([P, KT], F32, name="np_sb", tag="np_sb")
        nc.gpsimd.dma_start(out=np_sb, in_=nd[0].rearrange("(t p) -> p t", p=P))
        nc.vector.tensor_scalar_add(np_sb, np_sb, EPS)
        r1p = const.tile([P, KT], F32)
        nc.vector.reciprocal_approx_fast(r1p, np_sb)
        r1p_v = const.tile([P, KT], F32)     # Vn drain scale: r1 * 16/512
        nc.vector.tensor_scalar_mul(r1p_v, r1p, 1.0 / 32.0)
        # broadcast r1 over partitions [P, T] bf16 (folded into KT at drain)
        r1full = st1.tile([1, T], F32, name="r1f", tag="r1f")
        nc.vector.tensor_scalar_add(r1full, nrow, EPS)
        nc.vector.reciprocal_approx_fast(r1full, r1full)
        r1fb = st1.tile([1, T], BF16, name="r1fb", tag="r1fb")
        nc.vector.tensor_copy(out=r1fb, in_=r1full)
        r1fd = dram.tile([1, T], BF16, name="r1fd", tag="r1fd")
        nc.gpsimd.dma_start(out=r1fd, in_=r1fb)
        r1bc = const.tile([P, T], BF16)
        nc.gpsimd.dma_start(out=r1bc, in_=r1fd.to_broadcast([P, T]))

        # query-token stats from xqT (fp32) -> r512 [P, Q] broadcast
        ssqq = ssqp.tile([1, Q], F32, name="ssqq", tag="ssqq")
        for dt_ in range(DT):
            xqt = xqpool.tile([P, Q], F32, name="xqt", tag="xqt")
            nc.sync.dma_start(out=xqt, in_=ap["xqT"][dt_ * P:(dt_ + 1) * P, :])
            sq = sqpool.tile([P, Q], BF16, name="sqq", tag="sqq")
            nc.scalar.square(sq, xqt)
            nc.tensor.matmul(ssqq, lhsT=ones, rhs=sq,
                             start=(dt_ == 0), stop=(dt_ == DT - 1))
        nqrow = st1.tile([1, Q], F32)
        nc.scalar.activation(nqrow, ssqq, AF.Sqrt, scale=1.0 / D)
        nc.vector.tensor_scalar_add(nqrow, nqrow, EPS)
        r1row = st1.tile([1, Q], F32)
        nc.vector.reciprocal_approx_fast(r1row, nqrow)
        r1rd = dram.tile([1, Q], F32, name="r1rd", tag="r1rd")
        nc.scalar.dma_start(out=r1rd, in_=r1row)
        r512 = const.tile([P, Q], F32)
        nc.scalar.dma_start(out=r512, in_=r1rd.to_broadcast([P, Q]))

        # query x in fp8 pairs (host-provided)
        xqf = []
        for dp in range(DP):
            t8 = kvq.tile([P, 2, Q], F8, name=f"xqf{dp}", tag=f"xqf{dp}")
            nc.sync.dma_start(out=t8, in_=ap["xqf8"][dp])
            xqf.append(t8)
        ph1.close()

        # =============== Phase 2: K/V/Q projections (fp8 DR) ===============
        ph2 = ExitStack()
        bpool = ph2.enter_context(tc.tile_pool(name="b1", bufs=1))
        wq2pool = ph2.enter_context(tc.tile_pool(name="wq2p", bufs=3))

        # destination tiles (live through attention)
        KTs = [kvq.tile([P, T], F8, name=f"KT{hd}", tag=f"KT{hd}")
               for hd in range(HKV)]
        Vnp = [kvq.tile([P, 2, HKV * DH], F8, name=f"Vp{tp}", tag=f"Vp{tp}")
               for tp in range(KT // 2)]
        QTp = [kvq.tile([P, 2, Q], F8, name=f"QT{hp}", tag=f"QT{hp}")
               for hp in range(HQ // 2)]
        KW = {}
        for hd in range(HKV):
            w = wq2pool.tile([P, RKT, P], F8, name="wk2", tag=f"wk2{hd}")
            nc.scalar.dma_start(out=w, in_=ap["k2p8"][hd])
            KW[hd] = w

        # B1k/B1v: [P, RKT, T] f8 (stores 8x)
        B1 = {}
        for nm, wlist in (("k", w_k1), ("v", w_v1)):
            bt = bpool.tile([P, RKT, T], F8, name=f"B1{nm}", tag=f"B1{nm}")
            B1[nm] = bt
            for c in range(4):
                for rt in range(RKT):
                    pst = psA.tile([P, 512], F32, name="ps", tag="ps")
                    for dp in range(DP):
                        nc.tensor.matmul(pst, lhsT=wlist[rt][:, 2 * dp:2 * dp + 2, :],
                                         rhs=xf[dp][:, :, c * 512:(c + 1) * 512],
                                         start=(dp == 0), stop=(dp == DP - 1),
                                         perf_mode=DR)
                    nc.vector.tensor_scalar_mul(
                        bt[:, rt, c * 512:(c + 1) * 512], pst, 1.0 / 8.0)
                if nm == "k":
                    # KT tiles for this 512-token chunk
                    for hd in range(HKV):
                        pst = psA.tile([P, 512], F32, name="ps", tag="ps")
                        nc.tensor.matmul(pst, lhsT=KW[hd],
                                         rhs=bt[:, :, c * 512:(c + 1) * 512],
                                         start=True, stop=True, perf_mode=DR)
                        nc.vector.scalar_tensor_tensor(
                            KTs[hd][:, c * 512:(c + 1) * 512], in0=pst,
                            scalar=1.0 / 32.0, in1=r1bc[:, c * 512:(c + 1) * 512],
                            op0=AL.mult, op1=AL.mult)
                else:
                    # Vn pair tiles for this chunk's 4 token-tiles
                    for t in range(4 * c, 4 * c + 4):
                        pst = psA.tile([P, 512], F32, name="ps", tag="ps")
                        nc.tensor.matmul(pst, lhsT=bt[:, :, t * P:(t + 1) * P],
                                         rhs=v2sb,
                                         start=True, stop=True, perf_mode=DR)
                        nc.vector.tensor_scalar_mul(
                            Vnp[t // 2][:, t % 2, :], pst, r1p_v[:, t:t + 1])

        # A1: [P, 2, Q] f8 pair tiles (stores 8x, r1q applied)
        A1p = []
        for rp in range(RQT // 2):
            a = bpool.tile([P, 2, Q], F8, name=f"A1p{rp}", tag=f"A1p{rp}")
            A1p.append(a)
        for rt in range(RQT):
            w = wpool2.tile([P, DT, P], F8, name="wq1", tag="w8")
            nc.scalar.dma_start(out=w, in_=ap["q1p8"][rt])
            pst = psA.tile([P, 512], F32, name="ps", tag="ps")
            for dp in range(DP):
                nc.tensor.matmul(pst, lhsT=w[:, 2 * dp:2 * dp + 2, :], rhs=xqf[dp],
                                 start=(dp == 0), stop=(dp == DP - 1), perf_mode=DR)
            nc.vector.scalar_tensor_tensor(
                A1p[rt // 2][:, rt % 2, :], in0=pst, scalar=0.125, in1=r512,
                op0=AL.mult, op1=AL.mult)
        # QT pairs per head-pair: [P, 2, Q] f8 (stores 64x incl 1/sqrt(dh))
        for hd in range(HQ):
            w = wq2pool.tile([P, RQT, P], F8, name="wq2", tag="wq2")
            nc.scalar.dma_start(out=w, in_=ap["q2p8"][hd])
            pst = psA.tile([P, 512], F32, name="ps", tag="ps")
            for rp in range(RQT // 2):
                nc.tensor.matmul(pst, lhsT=w[:, 2 * rp:2 * rp + 2, :], rhs=A1p[rp],
                                 start=(rp == 0), stop=(rp == RQT // 2 - 1),
                                 perf_mode=DR)
            nc.vector.tensor_scalar_mul(QTp[hd // 2][:, hd % 2, :], pst, 0.125)
        ph2.close()
        wp2.close()
        pxf.close()
        psa_st.close()

        # =============== Phase 3: attention ===============
        ph3 = ExitStack()
        apool = ph3.enter_context(tc.tile_pool(name="att", bufs=3))
        mpool = ph3.enter_context(tc.tile_pool(name="mask", bufs=1))
        otrp = ph3.enter_context(tc.tile_pool(name="otr", bufs=2))
        spool = ph3.enter_context(tc.tile_pool(name="srow", bufs=2))
        plp = ph3.enter_context(tc.tile_pool(name="plp", bufs=2, space="PSUM"))
        pso = ph3.enter_context(tc.tile_pool(name="pso", bufs=1, space="PSUM"))
        pss = ph3.enter_context(tc.tile_pool(name="pss", bufs=1, space="PSUM"))

        expm_sb = {}
        for i in range(NCH):
            et = mpool.tile([P, 4, 2, P], BF16, name=f"em{i}", tag=f"em{i}")
            nc.gpsimd.dma_start(out=et, in_=ap["expm"][i])
            expm_sb[i] = et
        # preload act tables: Exp now (hidden under P2), Sqrt/Sigmoid later
        dact = spool.tile([1, 1], F32, name="dact", tag="dact")
        nc.scalar.activation(dact, ln16c[0:1, :], AF.Exp)

        wop_pre = []
        for dm in range(2):   # prefetch first Wo weight tiles during attention
            w = otpool.tile([P, DT, P], BF16, name=f"wopp{dm}", tag=f"wopp{dm}")
            nc.scalar.dma_start(out=w, in_=ap["wop"][dm])
            wop_pre.append(w)

        OTn = []   # per hp: [P, 2, NCH, P] bf16 normalized attention out
        for hp in range(HQ // 2):
            hk = hp // 2
            otx = otpool.tile([P, 2, NCH, P], BF16, name=f"OT{hp}", tag=f"OT{hp}")
            otraw = otrp.tile([P, 2, NCH, P], BF16, name="otr", tag="otr")
            s1row = spool.tile([1, 2, NCH * P], F32, name="srow", tag="srow")
            for i in range(NCH):
                po = pso.tile([P, 2, P], F32, name="po", tag="po")
                pS = pss.tile([32, 2, P], F32, name="pS", tag="pS")
                for g in range(i + 1):   # 4-kt groups; g == i is the diagonal
                    pl = plp.tile([P, 4, 2, P], F32, name="pl", tag="pl")
                    for s4 in range(4):
                        kt = 4 * g + s4
                        nc.tensor.matmul(pl[:, s4], lhsT=KTs[hk][:, kt * P:(kt + 1) * P],
                                         rhs=QTp[hp][:, :, i * P:(i + 1) * P],
                                         start=True, stop=True)
                    pt = apool.tile([P, 4, 2, P], F8, name="pt", tag="pt")
                    nc.scalar.activation(pt, pl, AF.Exp, scale=1.0 / 1024.0,
                                         bias=ln16c)
                    if g == i:   # diagonal 512-block: apply exp(mask)
                        nc.vector.tensor_tensor(pt, pt, expm_sb[i], AL.mult)
                    npair = 2 * (i + 1)
                    for p in range(2):
                        pp = 2 * g + p
                        nc.tensor.matmul(po, lhsT=Vnp[pp][:, :, hk * DH:(hk + 1) * DH],
                                         rhs=pt[:, 2 * p:2 * p + 2, :, :],
                                         start=(pp == 0), stop=(pp == npair - 1),
                                         perf_mode=DR)
                        nc.tensor.matmul(pS, lhsT=ones8,
                                         rhs=pt[:, 2 * p:2 * p + 2, :, :],
                                         start=(pp == 0), stop=(pp == npair - 1),
                                         perf_mode=DR)
                nc.vector.tensor_copy(out=otraw[:, :, i, :], in_=po)
                nc.vector.tensor_copy(out=s1row[:, :, i * P:(i + 1) * P], in_=pS[0:1])
            # normalization for this head pair (overlaps next hp's matmuls)
            sinv = spool.tile([1, 2, NCH * P], F32, name="sinv", tag="sinv")
            nc.vector.reciprocal_approx_fast(sinv, s1row)
            sinvb = spool.tile([1, 2, NCH * P], BF16, name="sinvb", tag="sinvb")
            nc.vector.tensor_copy(out=sinvb, in_=sinv)
            sbc = apool.tile([P, 2, NCH, P], BF16, name="sbc", tag="sbc")
            nc.gpsimd.partition_broadcast(sbc, sinvb)
            nc.vector.scalar_tensor_tensor(
                otx, in0=otraw, scalar=1.0 / 16.0,
                in1=sbc, op0=AL.mult, op1=AL.mult)
            OTn.append(otx)
        ph3.close()
        pkvq.close()

        # =============== Phase 4: Wo + residual + rmsnorm2 ===============
        x2pool = ctx.enter_context(tc.tile_pool(name="x2", bufs=1))
        h2pool = ctx.enter_context(tc.tile_pool(name="h2", bufs=1))
        psw_st = ExitStack()
        psW = psw_st.enter_context(tc.tile_pool(name="psW", bufs=2, space="PSUM"))
        ph4 = ExitStack()
        wpool = ph4.enter_context(tc.tile_pool(name="w4", bufs=3))
        st2 = ph4.enter_context(tc.tile_pool(name="st2", bufs=1))
        sq2pool = ph4.enter_context(tc.tile_pool(name="sq2", bufs=10))

        x2 = []
        ssq2 = None
        sq2_pend = []
        for dm in range(DT):
            if dm < 2:
                w = wop_pre[dm]
            else:
                w = wpool.tile([P, DT, P], BF16, name="w16", tag="w16")
                nc.sync.dma_start(out=w, in_=ap["wop"][dm])
            pst = psW.tile([P, 512], F32, name="ps", tag="ps")
            for din_ in range(DT):
                nc.tensor.matmul(pst, lhsT=w[:, din_, :],
                                 rhs=OTn[din_ // 2][:, din_ % 2, :, :],
                                 start=(din_ == 0), stop=(din_ == DT - 1))
            xqt = sq2pool.tile([P, Q], F32, name="xq4", tag="xq4")
            nc.scalar.dma_start(out=xqt, in_=ap["xqT"][dm * P:(dm + 1) * P, :])
            x2t = x2pool.tile([P, Q], F32, name=f"x2{dm}", tag=f"x2{dm}")
            nc.vector.tensor_tensor(x2t, pst, xqt, AL.add)
            x2.append(x2t)
            sq2 = sq2pool.tile([P, Q], BF16, name="sq2", tag="sq2")
            nc.scalar.square(sq2, x2t)
            sq2_pend.append(sq2)
            if dm == 8:
                # P3 attention PSUM pools have drained by now: safe to take 4
                # more banks. Delaying this alloc lets Wo start during P3 tail.
                ssq2p = ph4.enter_context(tc.tile_pool(name="ssq2p", bufs=1,
                                                       space="PSUM"))
                ssq2 = [ssq2p.tile([1, P], F32, name=f"ssq2_{r}", tag=f"ssq2_{r}")
                        for r in range(NCH)]
                dsq = st2.tile([1, 1], F32, name="dsq", tag="dsq")
                nc.scalar.activation(dsq, sq2[0:1, 0:1], AF.Sqrt)
            if ssq2 is not None:
                for pend_i, sqp in enumerate(sq2_pend):
                    for r in range(NCH):
                        nc.tensor.matmul(ssq2[r], lhsT=ones,
                                         rhs=sqp[:, r * P:(r + 1) * P],
                                         start=(dm == 8 and pend_i == 0),
                                         stop=(dm == DT - 1))
                sq2_pend = []

        s2row = st2.tile([1, Q], F32)
        for r in range(NCH):
            nc.vector.tensor_copy(out=s2row[:, r * P:(r + 1) * P], in_=ssq2[r])
        n2 = st2.tile([1, Q], F32)
        nc.scalar.activation(n2, s2row, AF.Sqrt, scale=1.0 / D)
        dsig = st2.tile([1, 1], F32, name="dsig", tag="dsig")
        nc.scalar.activation(dsig, n2[0:1, 0:1], AF.Sigmoid)  # preload for P5
        # EPS dropped: n2 >= 0.5 always, shifts r2 by < 2e-5 relative
        r2sb = st2.tile([1, Q], F32)
        nc.vector.reciprocal_approx_fast(r2sb, n2)
        r2b16 = st2.tile([1, Q], BF16)
        nc.vector.tensor_copy(out=r2b16, in_=r2sb)
        r2rep = st2.tile([P, Q], BF16)
        nc.gpsimd.partition_broadcast(r2rep, r2b16)
        h2bf = []
        h2f8 = [h2pool.tile([P, 2, Q], F8, name=f"h8{dp}", tag=f"h8{dp}")
                for dp in range(DP)]
        for dm in range(DT):
            h2t = h2pool.tile([P, Q], BF16, name=f"h2{dm}", tag=f"h2{dm}")
            nc.vector.tensor_tensor(h2t, x2[dm], r2rep, AL.mult)
            h2bf.append(h2t)
        for dm in range(DT):
            nc.vector.scalar_tensor_tensor(
                h2f8[dm // 2][:, dm % 2, :], in0=x2[dm], scalar=4.0, in1=r2rep,
                op0=AL.mult, op1=AL.mult)
        ph4.close()

        # =============== Phase 5: SwiGLU MLP + residual ===============
        ph5 = ExitStack()
        gpool = ph5.enter_context(tc.tile_pool(name="g", bufs=1))
        psb = ph5.enter_context(tc.tile_pool(name="psb", bufs=4, space="PSUM"))
        wpool5 = ph5.enter_context(tc.tile_pool(name="w5", bufs=4))
        spool5 = ph5.enter_context(tc.tile_pool(name="sig", bufs=3))
        wdpool = ph5.enter_context(tc.tile_pool(name="wd", bufs=2))
        opool = ph5.enter_context(tc.tile_pool(name="out", bufs=3))

        g = []          # f >= 16: bf16 tiles
        gf8 = [gpool.tile([P, 2, Q], F8, name=f"gf8{j}", tag=f"gf8{j}")
               for j in range(12)]  # f < 24: fp8 pair tiles
        for f in range(FT):
            wa = wpool5.tile([P, DT, P], BF16, name="w16", tag="w16")
            nc.sync.dma_start(out=wa, in_=ap["uap"][f])
            wb = wpool5.tile([P, DT, P], F8, name="w8b", tag="w8b")
            nc.sync.dma_start(out=wb, in_=ap["ubp8"][f])
            pa = psW.tile([P, 512], F32, name="ps", tag="ps")
            pb = psb.tile([P, 512], F32, name="psb", tag="psb")
            for i in range(DT):
                nc.tensor.matmul(pa, lhsT=wa[:, i, :], rhs=h2bf[i],
                                 start=(i == 0), stop=(i == DT - 1))
            for dp in range(DP):
                nc.tensor.matmul(pb, lhsT=wb[:, 2 * dp:2 * dp + 2, :], rhs=h2f8[dp],
                                 start=(dp == 0), stop=(dp == DP - 1), perf_mode=DR)
            sig = spool5.tile([P, Q], BF16, name="sig", tag="sig")
            nc.scalar.activation(sig, pb, AF.Sigmoid, scale=1.0 / 256.0)
            if f < 24:
                nc.vector.tensor_tensor(gf8[f // 2][:, f % 2, :], pa, sig, AL.mult)
            else:
                gt = gpool.tile([P, Q], BF16, name=f"g{f}", tag=f"g{f}")
                nc.vector.tensor_tensor(gt, pa, sig, AL.mult)
                g.append(gt)

        NB = FT - 24    # 40 bf16 f-tiles
        H = NB // 2
        for dm in range(DT):
            wd8 = wdpool.tile([P, 24, P], F8, name="wd8", tag="wd8")
            nc.sync.dma_start(out=wd8, in_=ap["wdp8"][dm])
            wd0 = wdpool.tile([P, H, P], BF16, name="wd", tag="wd")
            nc.sync.dma_start(out=wd0, in_=ap["wdp"][dm, :, 0:H, :])
            wd1 = wdpool.tile([P, H, P], BF16, name="wd", tag="wd")
            nc.sync.dma_start(out=wd1, in_=ap["wdp"][dm, :, H:NB, :])
            pst = psW.tile([P, 512], F32, name="ps", tag="ps")
            for j in range(12):
                nc.tensor.matmul(pst, lhsT=wd8[:, 2 * j:2 * j + 2, :], rhs=gf8[j],
                                 start=(j == 0), stop=False, perf_mode=DR)
            for fb in range(NB):
                wd = wd0 if fb < H else wd1
                nc.tensor.matmul(pst, lhsT=wd[:, fb % H, :], rhs=g[fb],
                                 start=False, stop=(fb == NB - 1))
            ot = opool.tile([P, Q], F32, name="outt", tag="outt")
            nc.vector.scalar_tensor_tensor(ot, in0=pst, scalar=1.0 / 64.0,
                                           in1=x2[dm], op0=AL.mult, op1=AL.add)
            nc.sync.dma_start(out=outT[dm * P:(dm + 1) * P, :], in_=ot)
        ph5.close()
        psw_st.close()

    nc.compile()
    return nc


def _pack_lhsT(w, dtype, scale=1.0):
    """[K, M] -> [M/128, 128, K/128, 128]: out[mt, p, kt, c] = w[kt*128+p, mt*128+c]."""
    K, M = w.shape
    kt, mt = K // P, M // P
    return np.ascontiguousarray(
        (w * scale).reshape(kt, P, mt, P).transpose(2, 1, 0, 3)).astype(dtype)


def prepare_in_maps(inputs):
    x = np.asarray(inputs["x"], np.float32)
    mask = np.asarray(inputs["attn_mask"], np.float32)[0, 0]          # [T, T]
    w1 = np.asarray(inputs["norm1_w"], np.float32)[:, None]
    w2 = np.asarray(inputs["norm2_w"], np.float32)[:, None]

    shared = {
        "q1p8": _pack_lhsT(w1 * np.asarray(inputs["Wq1"], np.float32), F8NP, WS),
        "q2p8": _pack_lhsT(np.asarray(inputs["Wq2"], np.float32) / math.sqrt(DH),
                           F8NP, WS),
        "k1p8": _pack_lhsT(w1 * np.asarray(inputs["Wk1"], np.float32), F8NP, WS),
        "k2p8": _pack_lhsT(np.asarray(inputs["Wk2"], np.float32), F8NP, WS),
        "v1p8": _pack_lhsT(w1 * np.asarray(inputs["Wv1"], np.float32), F8NP, WS),
        "v2n8": np.ascontiguousarray(
            (np.asarray(inputs["Wv2"], np.float32) * WS)
            .reshape(RKT, P, HKV * DH).transpose(1, 0, 2)).astype(F8NP),
        "wop": _pack_lhsT(np.asarray(inputs["Wo"], np.float32), BF),
        "uap": _pack_lhsT(w2 * np.asarray(inputs["W_upA"], np.float32), BF),
        "ubp8": _pack_lhsT(w2 * np.asarray(inputs["W_upB"], np.float32), F8NP, WS),
        "wdp": _pack_lhsT(np.asarray(inputs["W_down"], np.float32)[24 * P:] * WS,
                          BF),
        "wdp8": _pack_lhsT(np.asarray(inputs["W_down"], np.float32)[:24 * P] * WS,
                           F8NP, 1.0),
    }

    in_maps = []
    qq = np.arange(Q)
    for c in range(NCORES):
        b, j = c // 4, c % 4
        xT = np.ascontiguousarray(x[b].T)                              # [D, T]
        xf8 = np.ascontiguousarray(
            xT.reshape(DP, 2, P, T).transpose(0, 2, 1, 3)).astype(F8NP)
        qtok = (qq // P) * 512 + P * j + (qq % P)                      # [Q]
        xq = np.ascontiguousarray(x[b][qtok, :].T)                     # [D, Q] f32
        xqf8 = np.ascontiguousarray(
            xq.reshape(DP, 2, P, Q).transpose(0, 2, 1, 3)).astype(F8NP)
        # expm [i][k, t4, z, q] = exp(mask[qtok(i,q), 512i+128*t4+k])
        em = np.empty((NCH, P, 4, 2, P), np.float32)
        for i in range(NCH):
            rows = qtok[i * P:(i + 1) * P]                             # [128]
            blk = mask[rows][:, 512 * i:512 * (i + 1)]                 # [q, 512]
            e = np.exp(blk)                                            # [q, 512]
            e4 = e.reshape(P, 4, P).transpose(2, 1, 0)                 # [k, t4, q]
            em[i, :, :, 0, :] = e4
            em[i, :, :, 1, :] = e4
        m = dict(shared)
        m["xf8"] = xf8
        m["xqT"] = xq
        m["xqf8"] = xqf8
        m["expm"] = em.astype(BF)
        in_maps.append(m)
    return in_maps


def kernel(x, attn_mask, norm1_w, norm2_w, Wq1, Wq2, Wk1, Wk2, Wv1, Wv2, Wo,
           W_upA, W_upB, W_down):
    if "nc" not in _CACHE:
        _CACHE["nc"] = _build_nc()
    nc = _CACHE["nc"]

    inputs = dict(x=x, attn_mask=attn_mask, norm1_w=norm1_w, norm2_w=norm2_w,
                  Wq1=Wq1, Wq2=Wq2, Wk1=Wk1, Wk2=Wk2, Wv1=Wv1, Wv2=Wv2, Wo=Wo,
                  W_upA=W_upA, W_upB=W_upB, W_down=W_down)
    in_maps = prepare_in_maps(inputs)
    res = run_bass_kernel_spmd(nc, in_maps, core_ids=list(range(NCORES)))
    _CACHE["last_result"] = res

    out = np.empty((B, T, D), np.float32)
    qq = np.arange(Q)
    for c in range(NCORES):
        b, j = c // 4, c % 4
        qtok = (qq // P) * 512 + P * j + (qq % P)
        out[b, qtok, :] = res.results[c]["outT"].T
    return out


# revision 39
# speedup vs baseline: 1.0249x; 1.0249x over previous
"""Trainium2 Bass kernel for a dense transformer block (MLA attention + SwiGLU MLP).

Problem: B=2, T=2048, D=2048, HQ=16, HKV=4, DH=128, RQ=512, RKV=256, DFF=8192.

Sharding: sequence-parallel over 8 cores. Core c (batch b=c//4, lane j=c%4)
owns 4 query chunks of 128 tokens: chunk i = tokens [512i+128j, 512i+128(j+1)).
K/V are computed for the batch's full 2048 tokens on every core (replicated,
no collectives). Causal structure is exploited at 512-key granularity: chunk i
attends exactly keys [0, 512(i+1)) — identical loop structure on every core
(SPMD), with the diagonal 512-block handled by a host-provided exp(mask)
multiplier. This removes 37.5% of attention work vs full-T.

fp8 (e4m3) with DoubleRow matmuls (K=256/instr, 2x throughput) is used where
quantization noise is damped below the error budget:
  - Q/K/V low-rank projections (noise washed out by softmax averaging)
  - P*V and softmax-sum S (P in [0,1] scaled by 16 via Exp bias=ln 16)
  - W_upB up-projection (sigmoid damps its error by ~4x)
  - the first 16 of 64 W_down row-tiles (rest bf16; full-fp8 busts the gate)
W_upA, Wo and the rest of W_down stay bf16. Measured rel err 1.61e-2 < 2e-2.

Scales: all fp8 weights x64. x/xq f8 direct. B1/A1 store 8x, QT 64x, KT 16x,
Vn 16x (with r1), pt = 16*exp(l) via bias, h2f8 4x. Descale folded into
activation scales / drain multiplies; S and P*V scales cancel in O/S except a
1/16 folded into the normalization multiply.
"""
import math
import numpy as np
import ml_dtypes

import concourse.bass as bass
import concourse.mybir as mybir
import concourse.tile as tile
from concourse import bacc
from concourse.bass_utils import run_bass_kernel_spmd
from contextlib import ExitStack

B, T, D = 2, 2048, 2048
HQ, HKV, DH = 16, 4, 128
RQ, RKV = 512, 256
DFF = 8192
EPS = 1e-5
NCORES = 8
Q = 512          # queries per core (4 chunks of 128)
P = 128
DT = D // P      # 16 d tiles
DP = DT // 2     # 8 d-tile pairs
KT = T // P      # 16 key tiles
RQT = RQ // P    # 4
RKT = RKV // P   # 2
FT = DFF // P    # 64 dff tiles
GROUP = HQ // HKV
NCH = 4          # query chunks per core

F32 = mybir.dt.float32
BF16 = mybir.dt.bfloat16
F8 = mybir.dt.float8e4
BF = ml_dtypes.bfloat16
F8NP = ml_dtypes.float8_e4m3
DR = mybir.MatmulPerfMode.DoubleRow

WS = 64.0        # fp8 weight scale
LN16 = math.log(16.0)

_CACHE = {}


def _build_nc():
    nc = bacc.Bacc("TRN2", debug=False, num_devices=NCORES)
    ap = {}
    def din(name, shape, dt):
        ap[name] = nc.dram_tensor(name, list(shape), dt, kind="ExternalInput").ap()
    din("xf8", [DP, P, 2, T], F8)          # [dp][p, s, t] = x[t, 256dp+128s+p]
    din("xqT", [D, Q], F32)                # core's 512 query tokens, transposed
    din("xqf8", [DP, P, 2, Q], F8)
    din("expm", [NCH, P, 4, 2, P], BF16)   # [i][k, t4, z, q]
    din("q1p8", [RQT, P, DT, P], F8)
    din("q2p8", [HQ, P, RQT, P], F8)
    din("k1p8", [RKT, P, DT, P], F8)
    din("k2p8", [HKV, P, RKT, P], F8)
    din("v1p8", [RKT, P, DT, P], F8)
    din("v2n8", [P, RKT, HKV * DH], F8)
    din("wop", [DT, P, DT, P], BF16)
    din("uap", [FT, P, DT, P], BF16)
    din("ubp8", [FT, P, DT, P], F8)
    din("wdp", [DT, P, FT - 24, P], BF16)
    din("wdp8", [DT, P, 24, P], F8)
    outT = nc.dram_tensor("outT", [D, Q], F32, kind="ExternalOutput").ap()

    AL = mybir.AluOpType
    AF = mybir.ActivationFunctionType

    with tile.TileContext(nc) as tc, ExitStack() as ctx:
        const = ctx.enter_context(tc.tile_pool(name="const", bufs=1))
        dram = ctx.enter_context(tc.tile_pool(name="drsc", bufs=1, space="DRAM"))

        ones = const.tile([P, 1], BF16)
        nc.vector.memset(ones, 1.0)
        ones8 = const.tile([P, 2, 32], F8)  # 32 identical cols: DR ldweights
        nc.vector.memset(ones8, 1.0)        # rejects narrower APs
        ln16c = const.tile([P, 1], F32)
        nc.vector.memset(ln16c, LN16)

        otpool = ctx.enter_context(tc.tile_pool(name="ot", bufs=1))
        pkvq = ExitStack()
        kvq = pkvq.enter_context(tc.tile_pool(name="kvq", bufs=1))
        pxf = ExitStack()
        xfpool = pxf.enter_context(tc.tile_pool(name="xf", bufs=1))
        psa_st = ExitStack()
        psA = psa_st.enter_context(tc.tile_pool(name="psA", bufs=3, space="PSUM"))

        # =============== Phase 1: load x, stats ===============
        # weights first on the scalar queue so they arrive before P2 needs them
        wp2 = ExitStack()
        wpool2 = wp2.enter_context(tc.tile_pool(name="w2", bufs=3))
        w_k1 = []
        for rt in range(RKT):
            w = wpool2.tile([P, DT, P], F8, name="wk1", tag="w8")
            nc.scalar.dma_start(out=w, in_=ap["k1p8"][rt])
            w_k1.append(w)
        w_v1 = []
        for rt in range(RKT):
            w = wpool2.tile([P, DT, P], F8, name="wv1", tag="w8")
            nc.scalar.dma_start(out=w, in_=ap["v1p8"][rt])
            w_v1.append(w)
        v2sb = kvq.tile([P, RKT, HKV * DH], F8, name="v2", tag="v2")
        nc.scalar.dma_start(out=v2sb, in_=ap["v2n8"])

        ph1 = ExitStack()
        sqpool = ph1.enter_context(tc.tile_pool(name="sq", bufs=3))
        st1 = ph1.enter_context(tc.tile_pool(name="st1", bufs=1))
        ssqp = ph1.enter_context(tc.tile_pool(name="ssqp", bufs=1, space="PSUM"))
        xqpool = ph1.enter_context(tc.tile_pool(name="xq", bufs=3))

        # xf8 tiles stay resident through P2
        xf = []
        for dp in range(DP):
            xt = xfpool.tile([P, 2, T], F8, name=f"xf{dp}", tag=f"xf{dp}")
            nc.sync.dma_start(out=xt, in_=ap["xf8"][dp])
            xf.append(xt)
        # batch-wide sum of squares -> r1p [P, KT]
        ssq = [ssqp.tile([1, 512], F32, name=f"ssq{c}", tag=f"ssq{c}") for c in range(4)]
        for dp in range(DP):
            for s in range(2):
                i = 2 * dp + s
                sq = sqpool.tile([P, T], BF16, name="sq", tag="sq")
                if i % 4 == 3:
                    nc.vector.tensor_tensor(sq, xf[dp][:, s, :], xf[dp][:, s, :],
                                            AL.mult)
                else:
                    nc.scalar.square(sq, xf[dp][:, s, :])
                for c in range(4):
                    nc.tensor.matmul(ssq[c], lhsT=ones, rhs=sq[:, c * 512:(c + 1) * 512],
                                     start=(i == 0), stop=(i == DT - 1))
        nrow = st1.tile([1, T], F32)
        for c in range(4):
            nc.scalar.activation(nrow[:, c * 512:(c + 1) * 512], ssq[c],
                                 AF.Sqrt, scale=1.0 / D)
        nd = dram.tile([1, T], F32, name="r1nd", tag="r1nd")
        nc.gpsimd.dma_start(out=nd, in_=nrow)
        np_sb = st1.tile([P, KT], F32, name="np_sb", tag="np_sb")
        nc.gpsimd.dma_start(out=np_sb, in_=nd[0].rearrange("(t p) -> p t", p=P))
        nc.vector.tensor_scalar_add(np_sb, np_sb, EPS)
        r1p = const.tile([P, KT], F32)
        nc.vector.reciprocal_approx_fast(r1p, np_sb)
        r1p_v = const.tile([P, KT], F32)     # Vn drain scale: r1 * 16/512
        nc.vector.tensor_scalar_mul(r1p_v, r1p, 1.0 / 32.0)
        # broadcast r1 over partitions [P, T] bf16 (folded into KT at drain)
        r1full = st1.tile([1, T], F32, name="r1f", tag="r1f")
        nc.vector.tensor_scalar_add(r1full, nrow, EPS)
        nc.vector.reciprocal_approx_fast(r1full, r1full)
        r1fb = st1.tile([1, T], BF16, name="r1fb", tag="r1fb")
        nc.vector.tensor_copy(out=r1fb, in_=r1full)
        r1fd = dram.tile([1, T], BF16, name="r1fd", tag="r1fd")
        nc.gpsimd.dma_start(out=r1fd, in_=r1fb)
        r1bc = const.tile([P, T], BF16)
        nc.gpsimd.dma_start(out=r1bc, in_=r1fd.to_broadcast([P, T]))

        # query-token stats from xqT (fp32) -> r512 [P, Q] broadcast
        ssqq = ssqp.tile([1, Q], F32, name="ssqq", tag="ssqq")
        for dt_ in range(DT):
            xqt = xqpool.tile([P, Q], F32, name="xqt", tag="xqt")
            nc.sync.dma_start(out=xqt, in_=ap["xqT"][dt_ * P:(dt_ + 1) * P, :])
            sq = sqpool.tile([P, Q], BF16, name="sqq", tag="sqq")
            nc.scalar.square(sq, xqt)
            nc.tensor.matmul(ssqq, lhsT=ones, rhs=sq,
                             start=(dt_ == 0), stop=(dt_ == DT - 1))
        nqrow = st1.tile([1, Q], F32)
        nc.scalar.activation(nqrow, ssqq, AF.Sqrt, scale=1.0 / D)
        nc.vector.tensor_scalar_add(nqrow, nqrow, EPS)
        r1row = st1.tile([1, Q], F32)
        nc.vector.reciprocal_approx_fast(r1row, nqrow)
        r1rd = dram.tile([1, Q], F32, name="r1rd", tag="r1rd")
        nc.scalar.dma_start(out=r1rd, in_=r1row)
        r512 = const.tile([P, Q], F32)
        nc.scalar.dma_start(out=r512, in_=r1rd.to_broadcast([P, Q]))

        # query x in fp8 pairs (host-provided)
        xqf = []
        for dp in range(DP):
            t8 = kvq.tile([P, 2, Q], F8, name=f"xqf{dp}", tag=f"xqf{dp}")
            nc.sync.dma_start(out=t8, in_=ap["xqf8"][dp])
            xqf.append(t8)
        ph1.close()

        # =============== Phase 2: K/V/Q projections (fp8 DR) ===============
        ph2 = ExitStack()
        bpool = ph2.enter_context(tc.tile_pool(name="b1", bufs=1))
        wq2pool = ph2.enter_context(tc.tile_pool(name="wq2p", bufs=3))

        # destination tiles (live through attention)
        KTs = [kvq.tile([P, T], F8, name=f"KT{hd}", tag=f"KT{hd}")
               for hd in range(HKV)]
        Vnp = [kvq.tile([P, 2, HKV * DH], F8, name=f"Vp{tp}", tag=f"Vp{tp}")
               for tp in range(KT // 2)]
        QTp = [kvq.tile([P, 2, Q], F8, name=f"QT{hp}", tag=f"QT{hp}")
               for hp in range(HQ // 2)]
        KW = {}
        for hd in range(HKV):
            w = wq2pool.tile([P, RKT, P], F8, name="wk2", tag=f"wk2{hd}")
            nc.scalar.dma_start(out=w, in_=ap["k2p8"][hd])
            KW[hd] = w

        # B1k/B1v: [P, RKT, T] f8 (stores 8x)
        B1 = {}
        for nm, wlist in (("k", w_k1), ("v", w_v1)):
            bt = bpool.tile([P, RKT, T], F8, name=f"B1{nm}", tag=f"B1{nm}")
            B1[nm] = bt
            for c in range(4):
                for rt in range(RKT):
                    pst = psA.tile([P, 512], F32, name="ps", tag="ps")
                    for dp in range(DP):
                        nc.tensor.matmul(pst, lhsT=wlist[rt][:, 2 * dp:2 * dp + 2, :],
                                         rhs=xf[dp][:, :, c * 512:(c + 1) * 512],
                                         start=(dp == 0), stop=(dp == DP - 1),
                                         perf_mode=DR)
                    if nm == "k":
                        # r1 folded here (KT is linear in B1k columns), so the
                        # KT drain below is a plain scaled copy on scalar
                        nc.vector.scalar_tensor_tensor(
                            bt[:, rt, c * 512:(c + 1) * 512], in0=pst,
                            scalar=1.0 / 8.0, in1=r1bc[:, c * 512:(c + 1) * 512],
                            op0=AL.mult, op1=AL.mult)
                    else:
                        nc.vector.tensor_scalar_mul(
                            bt[:, rt, c * 512:(c + 1) * 512], pst, 1.0 / 8.0)
                if nm == "k":
                    # KT tiles for this 512-token chunk
                    for hd in range(HKV):
                        pst = psA.tile([P, 512], F32, name="ps", tag="ps")
                        nc.tensor.matmul(pst, lhsT=KW[hd],
                                         rhs=bt[:, :, c * 512:(c + 1) * 512],
                                         start=True, stop=True, perf_mode=DR)
                        nc.scalar.mul(KTs[hd][:, c * 512:(c + 1) * 512], pst,
                                      1.0 / 32.0)
                else:
                    # Vn pair tiles for this chunk's 4 token-tiles
                    for t in range(4 * c, 4 * c + 4):
                        pst = psA.tile([P, 512], F32, name="ps", tag="ps")
                        nc.tensor.matmul(pst, lhsT=bt[:, :, t * P:(t + 1) * P],
                                         rhs=v2sb,
                                         start=True, stop=True, perf_mode=DR)
                        nc.vector.tensor_scalar_mul(
                            Vnp[t // 2][:, t % 2, :], pst, r1p_v[:, t:t + 1])

        # A1: [P, 2, Q] f8 pair tiles (stores 8x, r1q applied)
        A1p = []
        for rp in range(RQT // 2):
            a = bpool.tile([P, 2, Q], F8, name=f"A1p{rp}", tag=f"A1p{rp}")
            A1p.append(a)
        for rt in range(RQT):
            w = wpool2.tile([P, DT, P], F8, name="wq1", tag="w8")
            nc.scalar.dma_start(out=w, in_=ap["q1p8"][rt])
            pst = psA.tile([P, 512], F32, name="ps", tag="ps")
            for dp in range(DP):
                nc.tensor.matmul(pst, lhsT=w[:, 2 * dp:2 * dp + 2, :], rhs=xqf[dp],
                                 start=(dp == 0), stop=(dp == DP - 1), perf_mode=DR)
            nc.vector.scalar_tensor_tensor(
                A1p[rt // 2][:, rt % 2, :], in0=pst, scalar=0.125, in1=r512,
                op0=AL.mult, op1=AL.mult)
        # QT pairs per head-pair: [P, 2, Q] f8 (stores 64x incl 1/sqrt(dh))
        for hd in range(HQ):
            w = wq2pool.tile([P, RQT, P], F8, name="wq2", tag="wq2")
            nc.scalar.dma_start(out=w, in_=ap["q2p8"][hd])
            pst = psA.tile([P, 512], F32, name="ps", tag="ps")
            for rp in range(RQT // 2):
                nc.tensor.matmul(pst, lhsT=w[:, 2 * rp:2 * rp + 2, :], rhs=A1p[rp],
                                 start=(rp == 0), stop=(rp == RQT // 2 - 1),
                                 perf_mode=DR)
            if hd % 2 == 0:
                nc.scalar.mul(QTp[hd // 2][:, hd % 2, :], pst, 0.125)
            else:
                nc.vector.tensor_scalar_mul(QTp[hd // 2][:, hd % 2, :], pst, 0.125)
        ph2.close()
        wp2.close()
        pxf.close()
        psa_st.close()

        # =============== Phase 3: attention ===============
        ph3 = ExitStack()
        apool = ph3.enter_context(tc.tile_pool(name="att", bufs=3))
        mpool = ph3.enter_context(tc.tile_pool(name="mask", bufs=1))
        otrp = ph3.enter_context(tc.tile_pool(name="otr", bufs=2))
        spool = ph3.enter_context(tc.tile_pool(name="srow", bufs=2))
        plp = ph3.enter_context(tc.tile_pool(name="plp", bufs=2, space="PSUM"))
        pso = ph3.enter_context(tc.tile_pool(name="pso", bufs=1, space="PSUM"))
        pss = ph3.enter_context(tc.tile_pool(name="pss", bufs=1, space="PSUM"))

        expm_sb = {}
        for i in range(NCH):
            et = mpool.tile([P, 4, 2, P], BF16, name=f"em{i}", tag=f"em{i}")
            nc.gpsimd.dma_start(out=et, in_=ap["expm"][i])
            expm_sb[i] = et
        # preload act tables: Exp now (hidden under P2), Sqrt/Sigmoid later
        dact = spool.tile([1, 1], F32, name="dact", tag="dact")
        nc.scalar.activation(dact, ln16c[0:1, :], AF.Exp)

        wop_pre = []
        for dm in range(2):   # prefetch first Wo weight tiles during attention
            w = otpool.tile([P, DT, P], BF16, name=f"wopp{dm}", tag=f"wopp{dm}")
            nc.scalar.dma_start(out=w, in_=ap["wop"][dm])
            wop_pre.append(w)

        OTn = []   # per hp: [P, 2, NCH, P] bf16 normalized attention out
        for hp in range(HQ // 2):
            hk = hp // 2
            otx = otpool.tile([P, 2, NCH, P], BF16, name=f"OT{hp}", tag=f"OT{hp}")
            otraw = otrp.tile([P, 2, NCH, P], BF16, name="otr", tag="otr")
            s_row = spool.tile([NCH, 2, P], F32, name="srow", tag="srow")
            for i in range(NCH):
                po = pso.tile([P, 2, P], F32, name="po", tag="po")
                pS = pss.tile([32, 2, P], F32, name="pS", tag="pS")
                for g in range(i + 1):   # 4-kt groups; g == i is the diagonal
                    pl = plp.tile([P, 4, 2, P], F32, name="pl", tag="pl")
                    for s4 in range(4):
                        kt = 4 * g + s4
                        nc.tensor.matmul(pl[:, s4], lhsT=KTs[hk][:, kt * P:(kt + 1) * P],
                                         rhs=QTp[hp][:, :, i * P:(i + 1) * P],
                                         start=True, stop=True)
                    pt = apool.tile([P, 4, 2, P], F8, name="pt", tag="pt")
                    nc.scalar.activation(pt, pl, AF.Exp, scale=1.0 / 1024.0,
                                         bias=ln16c)
                    if g == i:   # diagonal 512-block: apply exp(mask)
                        nc.vector.tensor_tensor(pt, pt, expm_sb[i], AL.mult)
                    npair = 2 * (i + 1)
                    for p in range(2):
                        pp = 2 * g + p
                        nc.tensor.matmul(po, lhsT=Vnp[pp][:, :, hk * DH:(hk + 1) * DH],
                                         rhs=pt[:, 2 * p:2 * p + 2, :, :],
                                         start=(pp == 0), stop=(pp == npair - 1),
                                         perf_mode=DR)
                        nc.tensor.matmul(pS, lhsT=ones8,
                                         rhs=pt[:, 2 * p:2 * p + 2, :, :],
                                         start=(pp == 0), stop=(pp == npair - 1),
                                         perf_mode=DR)
                nc.vector.tensor_copy(out=otraw[:, :, i, :], in_=po)
                s_tmp = apool.tile([1, 2, P], F32, name="stmp", tag="stmp")
                nc.vector.tensor_copy(out=s_tmp, in_=pS[0:1])
                nc.gpsimd.dma_start(out=s_row[i:i + 1], in_=s_tmp)
            # normalization for this head pair (overlaps next hp's matmuls)
            sinv = spool.tile([NCH, 2, P], F32, name="sinv", tag="sinv")
            nc.vector.reciprocal_approx_fast(sinv, s_row)
            sinvb = spool.tile([NCH, 2, P], BF16, name="sinvb", tag="sinvb")
            nc.vector.tensor_copy(out=sinvb, in_=sinv)
            sdram = dram.tile([1, 2 * NCH * P], BF16, name=f"sd{hp}", tag=f"sd{hp}")
            nc.gpsimd.dma_start(
                out=sdram.rearrange("o (z i q) -> i z q", z=2, i=NCH, q=P),
                in_=sinvb)
            sbc = apool.tile([P, 2, NCH, P], BF16, name="sbc", tag="sbc")
            nc.gpsimd.dma_start(out=sbc.rearrange("p a b c -> p (a b c)"),
                                in_=sdram.to_broadcast([P, 2 * NCH * P]))
            nc.vector.scalar_tensor_tensor(
                otx, in0=otraw, scalar=1.0 / 16.0,
                in1=sbc, op0=AL.mult, op1=AL.mult)
            OTn.append(otx)
        ph3.close()
        pkvq.close()

        # =============== Phase 4: Wo + residual + rmsnorm2 ===============
        x2pool = ctx.enter_context(tc.tile_pool(name="x2", bufs=1))
        h2pool = ctx.enter_context(tc.tile_pool(name="h2", bufs=1))
        psw_st = ExitStack()
        psW = psw_st.enter_context(tc.tile_pool(name="psW", bufs=2, space="PSUM"))
        ph4 = ExitStack()
        wpool = ph4.enter_context(tc.tile_pool(name="w4", bufs=3))
        st2 = ph4.enter_context(tc.tile_pool(name="st2", bufs=1))
        sq2pool = ph4.enter_context(tc.tile_pool(name="sq2", bufs=10))

        x2 = []
        ssq2 = None
        sq2_pend = []
        for dm in range(DT):
            if dm < 2:
                w = wop_pre[dm]
            else:
                w = wpool.tile([P, DT, P], BF16, name="w16", tag="w16")
                nc.sync.dma_start(out=w, in_=ap["wop"][dm])
            pst = psW.tile([P, 512], F32, name="ps", tag="ps")
            for din_ in range(DT):
                nc.tensor.matmul(pst, lhsT=w[:, din_, :],
                                 rhs=OTn[din_ // 2][:, din_ % 2, :, :],
                                 start=(din_ == 0), stop=(din_ == DT - 1))
            xqt = sq2pool.tile([P, Q], F32, name="xq4", tag="xq4")
            nc.scalar.dma_start(out=xqt, in_=ap["xqT"][dm * P:(dm + 1) * P, :])
            x2t = x2pool.tile([P, Q], F32, name=f"x2{dm}", tag=f"x2{dm}")
            nc.vector.tensor_tensor(x2t, pst, xqt, AL.add)
            x2.append(x2t)
            sq2 = sq2pool.tile([P, Q], BF16, name="sq2", tag="sq2")
            nc.scalar.square(sq2, x2t)
            sq2_pend.append(sq2)
            if dm == 8:
                # P3 attention PSUM pools have drained by now: safe to take 4
                # more banks. Delaying this alloc lets Wo start during P3 tail.
                ssq2p = ph4.enter_context(tc.tile_pool(name="ssq2p", bufs=1,
                                                       space="PSUM"))
                ssq2 = [ssq2p.tile([1, P], F32, name=f"ssq2_{r}", tag=f"ssq2_{r}")
                        for r in range(NCH)]
                dsq = st2.tile([1, 1], F32, name="dsq", tag="dsq")
                nc.scalar.activation(dsq, sq2[0:1, 0:1], AF.Sqrt)
            if ssq2 is not None:
                for pend_i, sqp in enumerate(sq2_pend):
                    for r in range(NCH):
                        nc.tensor.matmul(ssq2[r], lhsT=ones,
                                         rhs=sqp[:, r * P:(r + 1) * P],
                                         start=(dm == 8 and pend_i == 0),
                                         stop=(dm == DT - 1))
                sq2_pend = []

        s2row = st2.tile([1, Q], F32)
        for r in range(NCH):
            nc.vector.tensor_copy(out=s2row[:, r * P:(r + 1) * P], in_=ssq2[r])
        n2 = st2.tile([1, Q], F32)
        nc.scalar.activation(n2, s2row, AF.Sqrt, scale=1.0 / D)
        dsig = st2.tile([1, 1], F32, name="dsig", tag="dsig")
        nc.scalar.activation(dsig, n2[0:1, 0:1], AF.Sigmoid)  # preload for P5
        # EPS dropped: n2 >= 0.5 always, shifts r2 by < 2e-5 relative
        r2sb = st2.tile([1, Q], F32)
        nc.vector.reciprocal_approx_fast(r2sb, n2)
        r2b16 = st2.tile([1, Q], BF16)
        nc.vector.tensor_copy(out=r2b16, in_=r2sb)
        r2rd = dram.tile([1, Q], BF16, name="r2rd", tag="r2rd")
        nc.gpsimd.dma_start(out=r2rd, in_=r2b16)
        r2rep = st2.tile([P, Q], BF16)
        nc.gpsimd.dma_start(out=r2rep, in_=r2rd.to_broadcast([P, Q]))
        h2bf = []
        h2f8 = [h2pool.tile([P, 2, Q], F8, name=f"h8{dp}", tag=f"h8{dp}")
                for dp in range(DP)]
        for dm in range(DT):
            h2t = h2pool.tile([P, Q], BF16, name=f"h2{dm}", tag=f"h2{dm}")
            nc.vector.tensor_tensor(h2t, x2[dm], r2rep, AL.mult)
            h2bf.append(h2t)
        for dm in range(DT):
            nc.vector.scalar_tensor_tensor(
                h2f8[dm // 2][:, dm % 2, :], in0=x2[dm], scalar=4.0, in1=r2rep,
                op0=AL.mult, op1=AL.mult)
        ph4.close()

        # =============== Phase 5: SwiGLU MLP + residual ===============
        ph5 = ExitStack()
        gpool = ph5.enter_context(tc.tile_pool(name="g", bufs=1))
        psb = ph5.enter_context(tc.tile_pool(name="psb", bufs=4, space="PSUM"))
        wpool5 = ph5.enter_context(tc.tile_pool(name="w5", bufs=4))
        spool5 = ph5.enter_context(tc.tile_pool(name="sig", bufs=3))
        wdpool = ph5.enter_context(tc.tile_pool(name="wd", bufs=2))
        opool = ph5.enter_context(tc.tile_pool(name="out", bufs=3))

        g = []          # f >= 16: bf16 tiles
        gf8 = [gpool.tile([P, 2, Q], F8, name=f"gf8{j}", tag=f"gf8{j}")
               for j in range(12)]  # f < 24: fp8 pair tiles
        for f in range(FT):
            wa = wpool5.tile([P, DT, P], BF16, name="w16", tag="w16")
            nc.sync.dma_start(out=wa, in_=ap["uap"][f])
            wb = wpool5.tile([P, DT, P], F8, name="w8b", tag="w8b")
            nc.sync.dma_start(out=wb, in_=ap["ubp8"][f])
            pa = psW.tile([P, 512], F32, name="ps", tag="ps")
            pb = psb.tile([P, 512], F32, name="psb", tag="psb")
            for i in range(DT):
                nc.tensor.matmul(pa, lhsT=wa[:, i, :], rhs=h2bf[i],
                                 start=(i == 0), stop=(i == DT - 1))
            for dp in range(DP):
                nc.tensor.matmul(pb, lhsT=wb[:, 2 * dp:2 * dp + 2, :], rhs=h2f8[dp],
                                 start=(dp == 0), stop=(dp == DP - 1), perf_mode=DR)
            sig = spool5.tile([P, Q], BF16, name="sig", tag="sig")
            nc.scalar.activation(sig, pb, AF.Sigmoid, scale=1.0 / 256.0)
            if f < 24:
                nc.vector.tensor_tensor(gf8[f // 2][:, f % 2, :], pa, sig, AL.mult)
            else:
                gt = gpool.tile([P, Q], BF16, name=f"g{f}", tag=f"g{f}")
                nc.vector.tensor_tensor(gt, pa, sig, AL.mult)
                g.append(gt)

        NB = FT - 24    # 40 bf16 f-tiles
        H = NB // 2
        for dm in range(DT):
            wd8 = wdpool.tile([P, 24, P], F8, name="wd8", tag="wd8")
            nc.sync.dma_start(out=wd8, in_=ap["wdp8"][dm])
            wd0 = wdpool.tile([P, H, P], BF16, name="wd", tag="wd")
            nc.sync.dma_start(out=wd0, in_=ap["wdp"][dm, :, 0:H, :])
            wd1 = wdpool.tile([P, H, P], BF16, name="wd", tag="wd")
            nc.sync.dma_start(out=wd1, in_=ap["wdp"][dm, :, H:NB, :])
            pst = psW.tile([P, 512], F32, name="ps", tag="ps")
            for j in range(12):
                nc.tensor.matmul(pst, lhsT=wd8[:, 2 * j:2 * j + 2, :], rhs=gf8[j],
                                 start=(j == 0), stop=False, perf_mode=DR)
            for fb in range(NB):
                wd = wd0 if fb < H else wd1
                nc.tensor.matmul(pst, lhsT=wd[:, fb % H, :], rhs=g[fb],
                                 start=False, stop=(fb == NB - 1))
            ot = opool.tile([P, Q], F32, name="outt", tag="outt")
            nc.vector.scalar_tensor_tensor(ot, in0=pst, scalar=1.0 / 64.0,
                                           in1=x2[dm], op0=AL.mult, op1=AL.add)
            nc.sync.dma_start(out=outT[dm * P:(dm + 1) * P, :], in_=ot)
        ph5.close()
        psw_st.close()

    nc.compile()
    return nc


def _pack_lhsT(w, dtype, scale=1.0):
    """[K, M] -> [M/128, 128, K/128, 128]: out[mt, p, kt, c] = w[kt*128+p, mt*128+c]."""
    K, M = w.shape
    kt, mt = K // P, M // P
    return np.ascontiguousarray(
        (w * scale).reshape(kt, P, mt, P).transpose(2, 1, 0, 3)).astype(dtype)


def prepare_in_maps(inputs):
    x = np.asarray(inputs["x"], np.float32)
    mask = np.asarray(inputs["attn_mask"], np.float32)[0, 0]          # [T, T]
    w1 = np.asarray(inputs["norm1_w"], np.float32)[:, None]
    w2 = np.asarray(inputs["norm2_w"], np.float32)[:, None]

    shared = {
        "q1p8": _pack_lhsT(w1 * np.asarray(inputs["Wq1"], np.float32), F8NP, WS),
        "q2p8": _pack_lhsT(np.asarray(inputs["Wq2"], np.float32) / math.sqrt(DH),
                           F8NP, WS),
        "k1p8": _pack_lhsT(w1 * np.asarray(inputs["Wk1"], np.float32), F8NP, WS),
        "k2p8": _pack_lhsT(np.asarray(inputs["Wk2"], np.float32), F8NP, WS),
        "v1p8": _pack_lhsT(w1 * np.asarray(inputs["Wv1"], np.float32), F8NP, WS),
        "v2n8": np.ascontiguousarray(
            (np.asarray(inputs["Wv2"], np.float32) * WS)
            .reshape(RKT, P, HKV * DH).transpose(1, 0, 2)).astype(F8NP),
        "wop": _pack_lhsT(np.asarray(inputs["Wo"], np.float32), BF),
        "uap": _pack_lhsT(w2 * np.asarray(inputs["W_upA"], np.float32), BF),
        "ubp8": _pack_lhsT(w2 * np.asarray(inputs["W_upB"], np.float32), F8NP, WS),
        "wdp": _pack_lhsT(np.asarray(inputs["W_down"], np.float32)[24 * P:] * WS,
                          BF),
        "wdp8": _pack_lhsT(np.asarray(inputs["W_down"], np.float32)[:24 * P] * WS,
                           F8NP, 1.0),
    }

    in_maps = []
    qq = np.arange(Q)
    for c in range(NCORES):
        b, j = c // 4, c % 4
        xT = np.ascontiguousarray(x[b].T)                              # [D, T]
        xf8 = np.ascontiguousarray(
            xT.reshape(DP, 2, P, T).transpose(0, 2, 1, 3)).astype(F8NP)
        qtok = (qq // P) * 512 + P * j + (qq % P)                      # [Q]
        xq = np.ascontiguousarray(x[b][qtok, :].T)                     # [D, Q] f32
        xqf8 = np.ascontiguousarray(
            xq.reshape(DP, 2, P, Q).transpose(0, 2, 1, 3)).astype(F8NP)
        # expm [i][k, t4, z, q] = exp(mask[qtok(i,q), 512i+128*t4+k])
        em = np.empty((NCH, P, 4, 2, P), np.float32)
        for i in range(NCH):
            rows = qtok[i * P:(i + 1) * P]                             # [128]
            blk = mask[rows][:, 512 * i:512 * (i + 1)]                 # [q, 512]
            e = np.exp(blk)                                            # [q, 512]
            e4 = e.reshape(P, 4, P).transpose(2, 1, 0)                 # [k, t4, q]
            em[i, :, :, 0, :] = e4
            em[i, :, :, 1, :] = e4
        m = dict(shared)
        m["xf8"] = xf8
        m["xqT"] = xq
        m["xqf8"] = xqf8
        m["expm"] = em.astype(BF)
        in_maps.append(m)
    return in_maps


def kernel(x, attn_mask, norm1_w, norm2_w, Wq1, Wq2, Wk1, Wk2, Wv1, Wv2, Wo,
           W_upA, W_upB, W_down):
    if "nc" not in _CACHE:
        _CACHE["nc"] = _build_nc()
    nc = _CACHE["nc"]

    inputs = dict(x=x, attn_mask=attn_mask, norm1_w=norm1_w, norm2_w=norm2_w,
                  Wq1=Wq1, Wq2=Wq2, Wk1=Wk1, Wk2=Wk2, Wv1=Wv1, Wv2=Wv2, Wo=Wo,
                  W_upA=W_upA, W_upB=W_upB, W_down=W_down)
    in_maps = prepare_in_maps(inputs)
    res = run_bass_kernel_spmd(nc, in_maps, core_ids=list(range(NCORES)))
    _CACHE["last_result"] = res

    out = np.empty((B, T, D), np.float32)
    qq = np.arange(Q)
    for c in range(NCORES):
        b, j = c // 4, c % 4
        qtok = (qq // P) * 512 + P * j + (qq % P)
        out[b, qtok, :] = res.results[c]["outT"].T
    return out
